# revision 5
# baseline (speedup 1.0000x reference)
"""NF4-style 4-bit quantized linear: out = x @ dequant(w).T on 8 TRN2 NeuronCores.

Column-parallel sharding: core c owns output features [c*512, (c+1)*512) and the
corresponding contiguous slices of the packed weight + quant state arrays. x is
replicated. Each core:
  1. loads ALL quant state + packed weight chunks on the SWDGE (gpsimd) queue
     first, so nothing on the dequant critical path waits behind an xbar
     transpose (HWDGE transposes serialize against other HWDGE traffic),
  2. dequantizes its 512x4096 weight slice in 8 chunks of 4 k-tiles, each
     chunk's 6 DVE stages batched across all 4 o-tiles (one instr per stage),
  3. round-trips each chunk through DRAM with an xbar transpose DMA to get
     wT [k-partition, outf] layout,
  4. streams x through xbar transpose DMAs ([token, k] -> [k, token]) and runs
     the fp16 matmul on the PE array, accumulating in PSUM over 32 k-tiles.
The xbar chain is pinned [x0, w0, x1, w1, ...] (8 ramp x blocks of 128 tokens
interleaved with the 8 W chunks) so the first matmul is gated only by the first
1MB x block + first W chunk; ramp matmuls are emitted in chain-readiness order
so the PE never waits on a late chunk while an early one has work. Output
stores are batched per 512-token group. Host gathers per-core outputs.
"""
import numpy as np

import concourse.bass as bass
import concourse.mybir as mybir
import concourse.tile as tile
from concourse import bacc
from concourse.tile_rust import add_dep_helper as tile_rust_add_dep
from concourse.bass_utils import run_bass_kernel_spmd

F16 = mybir.dt.float16
F32 = mybir.dt.float32
I32 = mybir.dt.int32
Alu = mybir.AluOpType

P = 128
TOKENS = 8192
IN_F = 4096
OUT_F = 4096
N_CORES = 8
O_C = OUT_F // N_CORES          # 512 out features per core
KT = IN_F // P                  # 32 k-tiles
BPR = IN_F // 2                 # 2048 packed bytes per weight row
NB_O = O_C // P                 # 4 o-tiles of 128 rows
TB = 512                        # steady token block

NKC = 8                         # W chunks
KKC = KT // NKC                 # 4 k-tiles per chunk
KCW = IN_F // NKC               # 512 k values per chunk
BCC = BPR // NKC                # 256 packed bytes per chunk (per row)
NBC = BCC // 32                 # 8 quant blocks per chunk (per row)

RTB = 128                       # ramp token block (1 psum tile each)
NRB = 8                         # ramp blocks (8*128 = 1024 tokens = 8 psums)

# pinned xbar chain: x0 w0 x1 w1 ... x7 w7  (indices into the chain)
X_SLOT = [2 * i for i in range(NRB)]
W_SLOT = [2 * i + 1 for i in range(NKC)]


def _build(tokens=TOKENS):
    nc = bacc.Bacc("TRN2", target_bir_lowering=False, debug=False,
                   enable_asserts=False)

    x = nc.dram_tensor("x", [tokens, IN_F], F16, kind="ExternalInput").ap()
    qw = nc.dram_tensor("qw", [O_C, BPR], I32, kind="ExternalInput").ap()
    qam = nc.dram_tensor("qam", [O_C, 64], I32, kind="ExternalInput").ap()
    qcode = nc.dram_tensor("qcode", [O_C, 64], F32, kind="ExternalInput").ap()
    qoff = nc.dram_tensor("qoff", [O_C, 64], F32, kind="ExternalInput").ap()
    am2 = nc.dram_tensor("am2", [O_C, 16], F32, kind="ExternalInput").ap()
    c2 = nc.dram_tensor("c2", [O_C, 16], F32, kind="ExternalInput").ap()
    out = nc.dram_tensor("out", [tokens, O_C], F16, kind="ExternalOutput").ap()

    n_steady = (tokens - NRB * RTB) // TB

    with tile.TileContext(nc) as tc:
        with tc.tile_pool(name="wt_pool", bufs=1) as wt_pool, \
             tc.tile_pool(name="wdram", bufs=1, space="DRAM") as wdram, \
             tc.tile_pool(name="sc_pool", bufs=1) as sc_pool, \
             tc.tile_pool(name="dq", bufs=2) as dq, \
             tc.tile_pool(name="xt_pool", bufs=2) as xt_pool, \
             tc.tile_pool(name="ps_pool", bufs=8, space="PSUM") as ps_pool, \
             tc.tile_pool(name="ob_pool", bufs=1) as ob_pool:

            # ---- everything the dequant critical path needs goes on the
            # SWDGE (gpsimd) queue, issued before any xbar transpose: first
            # packed chunk, then all scale state, then the remaining chunks.
            qw3 = qw.rearrange("(a p) c -> p a c", p=P)
            qts = {}

            def load_chunk(kc):
                qt = dq.tile([P, NB_O, BCC], I32, name="qt", bufs=2)
                nc.gpsimd.dma_start(
                    qt, qw3[:, :, kc * BCC:(kc + 1) * BCC])
                qts[kc] = qt

            load_chunk(0)
            am3 = sc_pool.tile([P, NB_O, 64], F32, name="am3")
            nc.gpsimd.dma_start(am3, qam.rearrange("(a p) c -> p a c", p=P))
            cd3 = sc_pool.tile([P, NB_O, 64], F32, name="cd3")
            nc.gpsimd.dma_start(cd3, qcode.rearrange("(a p) c -> p a c", p=P))
            c23 = sc_pool.tile([P, NB_O, 16], F32, name="c23")
            nc.gpsimd.dma_start(c23, c2.rearrange("(a p) c -> p a c", p=P))
            am23 = sc_pool.tile([P, NB_O, 16], F32, name="am23")
            nc.gpsimd.dma_start(am23, am2.rearrange("(a p) c -> p a c", p=P))
            of3 = sc_pool.tile([P, NB_O, 64], F32, name="of3")
            nc.gpsimd.dma_start(of3, qoff.rearrange("(a p) c -> p a c", p=P))
            load_chunk(1)

            # ---- scale prep (DVE):  S = (am/code) * (am2/c2),  offS = off*S
            rc = sc_pool.tile([P, NB_O, 64], F32, name="rc")
            nc.vector.reciprocal(rc, cd3)
            s1 = sc_pool.tile([P, NB_O, 64], F32, name="s1")
            nc.vector.tensor_tensor(s1, am3, rc, Alu.mult)
            rc2 = sc_pool.tile([P, NB_O, 16], F32, name="rc2")
            nc.vector.reciprocal(rc2, c23)
            s2 = sc_pool.tile([P, NB_O, 16], F32, name="s2")
            nc.vector.tensor_tensor(s2, am23, rc2, Alu.mult)
            S3 = sc_pool.tile([P, NB_O, 64], F32, name="S3")
            nc.vector.tensor_tensor(
                S3, s1, s2.unsqueeze(3).broadcast_to([P, NB_O, 16, 4]), Alu.mult)
            offS3 = sc_pool.tile([P, NB_O, 64], F32, name="offS3")
            nc.vector.tensor_tensor(offS3, of3, S3, Alu.mult)

            # ---- dequant + W round-trip, chunk-major, ops batched over the
            # whole chunk (4 o-tiles at once) ----
            wts = []
            wt_insts = []
            for kc in range(NKC):
                if kc + 2 < NKC:
                    load_chunk(kc + 2)
                wd = wdram.tile([O_C, KCW], F16, name=f"wd{kc}")
                w_nat = dq.tile([P, NB_O, KCW], F16, name="wn", bufs=2)
                qt = qts[kc]
                hi = dq.tile([P, NB_O, BCC], I32, name="hi", bufs=1)
                nc.vector.tensor_scalar(hi, qt, 4, None,
                                        Alu.logical_shift_right)
                lo = dq.tile([P, NB_O, BCC], F16, name="lo")
                nc.vector.scalar_tensor_tensor(
                    lo, hi, -16.0, qt, Alu.mult, Alu.add)
                sb = S3[:, :, kc * NBC:(kc + 1) * NBC] \
                    .unsqueeze(3).broadcast_to([P, NB_O, NBC, 32])
                mlo = dq.tile([P, NB_O, BCC], F16, name="mlo")
                nc.vector.tensor_tensor(mlo, lo, sb, Alu.mult)
                mhi = dq.tile([P, NB_O, BCC], F16, name="mhi")
                nc.vector.tensor_tensor(mhi, hi, sb, Alu.mult)
                offs = offS3[:, :, kc * NBC:(kc + 1) * NBC] \
                    .unsqueeze(3).broadcast_to([P, NB_O, NBC, 32])
                nc.vector.tensor_tensor(w_nat[:, :, 0::2], mlo, offs,
                                        Alu.subtract)
                nc.vector.tensor_tensor(w_nat[:, :, 1::2], mhi, offs,
                                        Alu.subtract)
                qts.pop(kc)
                nc.gpsimd.dma_start(
                    wd[:, :].rearrange("(a p) c -> p a c", p=P), w_nat)
                wt = wt_pool.tile([P, KKC, O_C], F16, name=f"wt{kc}")
                wi = nc.scalar.dma_start(out=wt, in_=wd[:, :], transpose=True)
                wts.append(wt)
                wt_insts.append(wi)

            # ---- ramp x transposes (1MB blocks of 128 tokens) ----
            xtr, xtr_insts = [], []
            for rb in range(NRB):
                t = xt_pool.tile([P, KT, RTB], F16, name=f"xtr{rb}", bufs=1)
                ti = nc.scalar.dma_start(
                    out=t, in_=x[rb * RTB:(rb + 1) * RTB, :], transpose=True)
                xtr.append(t)
                xtr_insts.append(ti)

            # ---- ramp matmuls, emitted in chain-readiness order so the PE
            # never program-order-blocks on a late chunk/block ----
            rps = [ps_pool.tile([P, O_C], F32, name="ps") for rb in range(NRB)]
            groups = sorted(
                ((max(W_SLOT[kc], X_SLOT[rb]), kc, rb)
                 for kc in range(NKC) for rb in range(NRB)))
            for _, kc, rb in groups:
                for j in range(KKC):
                    kk = kc * KKC + j
                    nc.tensor.matmul(
                        rps[rb],
                        xtr[rb][:, kk, :],
                        wts[kc][:, j, :],
                        start=(kk == 0),
                        stop=(kk == KT - 1),
                    )
            for g in range(NRB * RTB // TB):
                ob = ob_pool.tile([P, TB // P, O_C], F16, name="ob")
                for i in range(TB // P):
                    nc.vector.tensor_copy(ob[:, i, :], rps[g * (TB // P) + i])
                r0 = g * TB
                nc.gpsimd.dma_start(
                    out[r0:r0 + TB, :].rearrange("(st p) c -> p st c", p=P),
                    ob)

            # ---- steady blocks ----
            base = NRB * RTB
            xt_insts = []
            for tb in range(n_steady):
                xt = xt_pool.tile([P, KT, TB], F16, name="xt")
                xi = nc.scalar.dma_start(
                    out=xt, in_=x[base + tb * TB: base + (tb + 1) * TB, :],
                    transpose=True)
                xt_insts.append(xi)
                ob = ob_pool.tile([P, TB // P, O_C], F16, name="ob")
                for st in range(TB // P):
                    ps = ps_pool.tile([P, O_C], F32, name="ps")
                    for kk in range(KT):
                        nc.tensor.matmul(
                            ps,
                            xt[:, kk, st * P:(st + 1) * P],
                            wts[kk // KKC][:, kk % KKC, :],
                            start=(kk == 0),
                            stop=(kk == KT - 1),
                        )
                    nc.vector.tensor_copy(ob[:, st, :], ps)
                r0 = base + tb * TB
                nc.gpsimd.dma_start(
                    out[r0:r0 + TB, :].rearrange("(st p) c -> p st c", p=P),
                    ob)

            # ---- pin the xbar ring order: interleave ramp x blocks and W
            # chunks 1:1 (each pair gates one diagonal of ramp matmuls),
            # then the steady x blocks.
            chain = [None] * (NRB + NKC)
            for rb in range(NRB):
                chain[X_SLOT[rb]] = xtr_insts[rb]
            for kc in range(NKC):
                chain[W_SLOT[kc]] = wt_insts[kc]
            chain += xt_insts
            for a, b in zip(chain[1:], chain):
                tile_rust_add_dep(a.ins, b.ins, True, "xbar order")

    nc.compile()
    return nc


_NC_CACHE = {}


def _get_nc(tokens=TOKENS):
    if tokens not in _NC_CACHE:
        _NC_CACHE[tokens] = _build(tokens)
    return _NC_CACHE[tokens]


def _shard(inputs):
    x = np.ascontiguousarray(np.asarray(inputs["x"], dtype=np.float16))
    qw = np.asarray(inputs["quantized_weight"], dtype=np.int32)
    qam = np.asarray(inputs["quant_absmax"], dtype=np.int32)
    qcode = np.asarray(inputs["quant_code"], dtype=np.float32)
    qoff = np.asarray(inputs["quant_offset"], dtype=np.float32)
    am2 = np.asarray(inputs["state2_absmax"], dtype=np.float32)
    c2 = np.asarray(inputs["state2_code"], dtype=np.float32)

    pb = O_C * BPR        # packed bytes per core
    nb1 = O_C * 64        # primary blocks per core
    nb2 = O_C * 16        # secondary blocks per core
    in_maps = []
    for c in range(N_CORES):
        in_maps.append({
            "x": x,
            "qw": np.ascontiguousarray(
                qw[c * pb:(c + 1) * pb].reshape(O_C, BPR)),
            "qam": np.ascontiguousarray(
                qam[c * nb1:(c + 1) * nb1].reshape(O_C, 64)),
            "qcode": np.ascontiguousarray(
                qcode[c * nb1:(c + 1) * nb1].reshape(O_C, 64)),
            "qoff": np.ascontiguousarray(
                qoff[c * nb1:(c + 1) * nb1].reshape(O_C, 64)),
            "am2": np.ascontiguousarray(
                am2[c * nb2:(c + 1) * nb2].reshape(O_C, 16)),
            "c2": np.ascontiguousarray(
                c2[c * nb2:(c + 1) * nb2].reshape(O_C, 16)),
        })
    return in_maps


def _run(inputs, trace=False, trace_cores=None):
    nc = _get_nc()
    in_maps = _shard(inputs)
    res = run_bass_kernel_spmd(
        nc, in_maps, list(range(N_CORES)), trace=trace,
        trace_cores=trace_cores)
    out = np.concatenate([r["out"] for r in res.results], axis=1)
    return out, res


def kernel(**inputs) -> np.ndarray:
    out, _ = _run(inputs, trace=False)
    return out


# revision 7
# speedup vs baseline: 1.0088x; 1.0088x over previous
"""NF4-style 4-bit quantized linear: out = x @ dequant(w).T on 8 TRN2 NeuronCores.

Column-parallel sharding: core c owns output features [c*512, (c+1)*512) and the
corresponding contiguous slices of the packed weight + quant state arrays; x is
replicated. All DMA traffic serializes against in-flight xbar transposes, so
the kernel is scheduled as one conveyor:
  1. packed-weight chunk loads (cast int32->int16 on the SWDGE queue) + scale
     state first; the first x transpose is pinned AFTER the last scale load so
     the dequant critical path is never stuck behind a 9us transpose,
  2. dequant runs on DVE in 8 chunks of 4 k-tiles, 6 batched 2-byte ops per
     chunk (int16 nibbles x fp16 scales) for ~2x DVE rate,
  3. each chunk round-trips through DRAM with an xbar transpose to reach
     wT [k-partition, outf] layout; the xbar chain is pinned
     [x0 w0 w1 x1 w2 w3 x2 w4 w5 x3 w6 w7 | transition | steady] so W is fully
     resident by ~70us while ramp x blocks keep the PE fed,
  4. ramp matmuls (4 blocks x 256 tokens, 8 psums) are emitted in
     chain-readiness order; 2 transition blocks of 256 tokens bridge into
     512-token steady blocks.
A train of warmup matmuls on zeroed tiles keeps the PE busy from ~7us so the
HAM clock gate is warm when real matmuls start. Output stores are batched per
block; the last block stores per 128-token tile to shorten the drain.
"""
import numpy as np

import concourse.bass as bass
import concourse.mybir as mybir
import concourse.tile as tile
from concourse import bacc
from concourse.tile_rust import add_dep_helper as tile_rust_add_dep
from concourse.bass_utils import run_bass_kernel_spmd

F16 = mybir.dt.float16
F32 = mybir.dt.float32
I32 = mybir.dt.int32
I16 = mybir.dt.int16
Alu = mybir.AluOpType

P = 128
TOKENS = 8192
IN_F = 4096
OUT_F = 4096
N_CORES = 8
O_C = OUT_F // N_CORES          # 512 out features per core
KT = IN_F // P                  # 32 k-tiles
BPR = IN_F // 2                 # 2048 packed bytes per weight row
NB_O = O_C // P                 # 4 o-tiles of 128 rows
TB = 512                        # steady token block

NKC = 8                         # W chunks
KKC = KT // NKC                 # 4 k-tiles per chunk
KCW = IN_F // NKC               # 512 k values per chunk
BCC = BPR // NKC                # 256 packed bytes per chunk (per row)
NBC = BCC // 32                 # 8 quant blocks per chunk (per row)

RTB = 256                       # ramp token block (2 psum tiles each)
NRB = 4                         # ramp blocks (4*256 = 1024 tokens = 8 psums)
NTRANS = 2                      # 256-token transition blocks after the ramp

N_WARM = 52                     # warmup matmuls (~11us) to warm the HAM gate

# pinned xbar chain slots: x0 w0 w1 x1 w2 w3 x2 w4 w5 x3 w6 w7
X_CH = [0, 3, 6, 9]
W_CH = [1, 2, 4, 5, 7, 8, 10, 11]


def _build(tokens=TOKENS):
    nc = bacc.Bacc("TRN2", target_bir_lowering=False, debug=False,
                   enable_asserts=False)

    x = nc.dram_tensor("x", [tokens, IN_F], F16, kind="ExternalInput").ap()
    qw = nc.dram_tensor("qw", [O_C, BPR], I32, kind="ExternalInput").ap()
    qam = nc.dram_tensor("qam", [O_C, 64], I32, kind="ExternalInput").ap()
    qcode = nc.dram_tensor("qcode", [O_C, 64], F32, kind="ExternalInput").ap()
    qoff = nc.dram_tensor("qoff", [O_C, 64], F32, kind="ExternalInput").ap()
    am2 = nc.dram_tensor("am2", [O_C, 16], F32, kind="ExternalInput").ap()
    c2 = nc.dram_tensor("c2", [O_C, 16], F32, kind="ExternalInput").ap()
    out = nc.dram_tensor("out", [tokens, O_C], F16, kind="ExternalOutput").ap()

    n_steady = (tokens - (NRB + NTRANS) * RTB) // TB

    with tile.TileContext(nc) as tc:
        with tc.tile_pool(name="wt_pool", bufs=1) as wt_pool, \
             tc.tile_pool(name="wdram", bufs=1, space="DRAM") as wdram, \
             tc.tile_pool(name="sc_pool", bufs=1) as sc_pool, \
             tc.tile_pool(name="dq", bufs=2) as dq, \
             tc.tile_pool(name="xt_pool", bufs=2) as xt_pool, \
             tc.tile_pool(name="ps_pool", bufs=8, space="PSUM") as ps_pool, \
             tc.tile_pool(name="ob_pool", bufs=1) as ob_pool:

            # ---- SWDGE loads: first packed chunk (cast to int16), then all
            # scale state, then the rest of the chunks. Nothing on the
            # dequant critical path waits behind an xbar transpose.
            qw3 = qw.rearrange("(a p) c -> p a c", p=P)
            qts = {}

            def load_chunk(kc):
                qt = dq.tile([P, NB_O, BCC], I16, name="qt", bufs=2)
                nc.gpsimd.dma_start(
                    qt, qw3[:, :, kc * BCC:(kc + 1) * BCC])
                qts[kc] = qt

            load_chunk(0)
            am3 = sc_pool.tile([P, NB_O, 64], F32, name="am3")
            nc.gpsimd.dma_start(am3, qam.rearrange("(a p) c -> p a c", p=P))
            cd3 = sc_pool.tile([P, NB_O, 64], F32, name="cd3")
            nc.gpsimd.dma_start(cd3, qcode.rearrange("(a p) c -> p a c", p=P))
            c23 = sc_pool.tile([P, NB_O, 16], F32, name="c23")
            nc.gpsimd.dma_start(c23, c2.rearrange("(a p) c -> p a c", p=P))
            am23 = sc_pool.tile([P, NB_O, 16], F32, name="am23")
            nc.gpsimd.dma_start(am23, am2.rearrange("(a p) c -> p a c", p=P))
            of3 = sc_pool.tile([P, NB_O, 64], F32, name="of3")
            last_load = nc.gpsimd.dma_start(
                of3, qoff.rearrange("(a p) c -> p a c", p=P))
            load_chunk(1)

            # ---- warmup matmuls on zeroed tiles: PE busy from ~7us so the
            # HAM clock gate is at 8/8 when real matmuls start.
            wz = sc_pool.tile([P, P], F16, name="wz")
            nc.vector.memset(wz, 0.0)
            ww = sc_pool.tile([P, O_C], F16, name="ww")
            nc.vector.memset(ww, 0.0)
            wps = ps_pool.tile([P, O_C], F32, name="ps")
            for _ in range(N_WARM):
                nc.tensor.matmul(wps, wz, ww, start=True, stop=True)

            # ---- scale prep (DVE):  S = (am/code) * (am2/c2) as fp16,
            # offS = off*S
            rc = sc_pool.tile([P, NB_O, 64], F32, name="rc")
            nc.vector.reciprocal(rc, cd3)
            s1 = sc_pool.tile([P, NB_O, 64], F32, name="s1")
            nc.vector.tensor_tensor(s1, am3, rc, Alu.mult)
            rc2 = sc_pool.tile([P, NB_O, 16], F32, name="rc2")
            nc.vector.reciprocal(rc2, c23)
            s2 = sc_pool.tile([P, NB_O, 16], F32, name="s2")
            nc.vector.tensor_tensor(s2, am23, rc2, Alu.mult)
            S3 = sc_pool.tile([P, NB_O, 64], F16, name="S3")
            nc.vector.tensor_tensor(
                S3, s1, s2.unsqueeze(3).broadcast_to([P, NB_O, 16, 4]), Alu.mult)
            offS3 = sc_pool.tile([P, NB_O, 64], F16, name="offS3")
            nc.vector.tensor_tensor(offS3, of3, S3, Alu.mult)

            # ---- dequant + W round-trip, chunk-major, 2-byte batched ops ----
            wts = []
            wt_insts = []
            for kc in range(NKC):
                if kc + 2 < NKC:
                    load_chunk(kc + 2)
                wd = wdram.tile([O_C, KCW], F16, name=f"wd{kc}")
                w_nat = dq.tile([P, NB_O, KCW], F16, name="wn", bufs=2)
                qt = qts[kc]
                hi = dq.tile([P, NB_O, BCC], I16, name="hi", bufs=1)
                nc.vector.tensor_scalar(hi, qt, 4, None,
                                        Alu.logical_shift_right)
                lo = dq.tile([P, NB_O, BCC], F16, name="lo")
                nc.vector.scalar_tensor_tensor(
                    lo, hi, -16.0, qt, Alu.mult, Alu.add)
                sb = S3[:, :, kc * NBC:(kc + 1) * NBC] \
                    .unsqueeze(3).broadcast_to([P, NB_O, NBC, 32])
                mlo = dq.tile([P, NB_O, BCC], F16, name="mlo")
                nc.vector.tensor_tensor(mlo, lo, sb, Alu.mult)
                mhi = dq.tile([P, NB_O, BCC], F16, name="mhi")
                nc.vector.tensor_tensor(mhi, hi, sb, Alu.mult)
                offs = offS3[:, :, kc * NBC:(kc + 1) * NBC] \
                    .unsqueeze(3).broadcast_to([P, NB_O, NBC, 32])
                nc.vector.tensor_tensor(w_nat[:, :, 0::2], mlo, offs,
                                        Alu.subtract)
                nc.vector.tensor_tensor(w_nat[:, :, 1::2], mhi, offs,
                                        Alu.subtract)
                qts.pop(kc)
                nc.gpsimd.dma_start(
                    wd[:, :].rearrange("(a p) c -> p a c", p=P), w_nat)
                wt = wt_pool.tile([P, KKC, O_C], F16, name=f"wt{kc}")
                wi = nc.scalar.dma_start(out=wt, in_=wd[:, :], transpose=True)
                wts.append(wt)
                wt_insts.append(wi)

            # ---- ramp x transposes (2MB blocks of 256 tokens) ----
            xtr, xtr_insts = [], []
            for rb in range(NRB):
                t = xt_pool.tile([P, KT, RTB], F16, name=f"xtr{rb}", bufs=1)
                ti = nc.scalar.dma_start(
                    out=t, in_=x[rb * RTB:(rb + 1) * RTB, :], transpose=True)
                xtr.append(t)
                xtr_insts.append(ti)

            # ---- ramp matmuls, emitted in chain-readiness order ----
            rps = [[ps_pool.tile([P, O_C], F32, name="ps")
                    for st in range(RTB // P)] for rb in range(NRB)]
            groups = sorted(
                ((max(W_CH[kc], X_CH[rb]), kc, rb)
                 for kc in range(NKC) for rb in range(NRB)))
            for _, kc, rb in groups:
                for st in range(RTB // P):
                    for j in range(KKC):
                        kk = kc * KKC + j
                        nc.tensor.matmul(
                            rps[rb][st],
                            xtr[rb][:, kk, st * P:(st + 1) * P],
                            wts[kc][:, j, :],
                            start=(kk == 0),
                            stop=(kk == KT - 1),
                        )
            for g in range(NRB * RTB // TB):
                ob = ob_pool.tile([P, TB // P, O_C], F16, name="ob")
                for i in range(TB // P):
                    rb, st = divmod(g * (TB // P) + i, RTB // P)
                    nc.vector.tensor_copy(ob[:, i, :], rps[rb][st])
                r0 = g * TB
                nc.gpsimd.dma_start(
                    out[r0:r0 + TB, :].rearrange("(st p) c -> p st c", p=P),
                    ob)

            # ---- transition (256-token) + steady (512-token) blocks ----
            blocks = []
            pos = NRB * RTB
            for _ in range(NTRANS):
                blocks.append((pos, RTB))
                pos += RTB
            for _ in range(n_steady):
                blocks.append((pos, TB))
                pos += TB
            assert pos == tokens

            xt_insts = []
            for bi, (r0, bt) in enumerate(blocks):
                nst = bt // P
                xt = xt_pool.tile([P, KT, bt], F16, name="xt")
                xi = nc.scalar.dma_start(
                    out=xt, in_=x[r0:r0 + bt, :], transpose=True)
                xt_insts.append(xi)
                last = (bi == len(blocks) - 1)
                ob = ob_pool.tile([P, nst, O_C], F16, name="ob")
                for st in range(nst):
                    ps = ps_pool.tile([P, O_C], F32, name="ps")
                    for kk in range(KT):
                        nc.tensor.matmul(
                            ps,
                            xt[:, kk, st * P:(st + 1) * P],
                            wts[kk // KKC][:, kk % KKC, :],
                            start=(kk == 0),
                            stop=(kk == KT - 1),
                        )
                    nc.vector.tensor_copy(ob[:, st, :], ps)
                    if last:
                        # store per 128-token tile to shorten the drain
                        nc.gpsimd.dma_start(
                            out[r0 + st * P:r0 + (st + 1) * P, :],
                            ob[:, st, :])
                if not last:
                    nc.gpsimd.dma_start(
                        out[r0:r0 + bt, :].rearrange("(st p) c -> p st c", p=P),
                        ob)

            # ---- pin the xbar ring order; first x block also waits for the
            # last scale load so dequant inputs never queue behind it.
            chain = [None] * (len(X_CH) + len(W_CH))
            for rb in range(NRB):
                chain[X_CH[rb]] = xtr_insts[rb]
            for kc in range(NKC):
                chain[W_CH[kc]] = wt_insts[kc]
            chain += xt_insts
            for a, b in zip(chain[1:], chain):
                tile_rust_add_dep(a.ins, b.ins, True, "xbar order")
            tile_rust_add_dep(chain[0].ins, last_load.ins, True, "scales first")

    nc.compile()
    return nc


_NC_CACHE = {}


def _get_nc(tokens=TOKENS):
    if tokens not in _NC_CACHE:
        _NC_CACHE[tokens] = _build(tokens)
    return _NC_CACHE[tokens]


def _shard(inputs):
    x = np.ascontiguousarray(np.asarray(inputs["x"], dtype=np.float16))
    qw = np.asarray(inputs["quantized_weight"], dtype=np.int32)
    qam = np.asarray(inputs["quant_absmax"], dtype=np.int32)
    qcode = np.asarray(inputs["quant_code"], dtype=np.float32)
    qoff = np.asarray(inputs["quant_offset"], dtype=np.float32)
    am2 = np.asarray(inputs["state2_absmax"], dtype=np.float32)
    c2 = np.asarray(inputs["state2_code"], dtype=np.float32)

    pb = O_C * BPR        # packed bytes per core
    nb1 = O_C * 64        # primary blocks per core
    nb2 = O_C * 16        # secondary blocks per core
    in_maps = []
    for c in range(N_CORES):
        in_maps.append({
            "x": x,
            "qw": np.ascontiguousarray(
                qw[c * pb:(c + 1) * pb].reshape(O_C, BPR)),
            "qam": np.ascontiguousarray(
                qam[c * nb1:(c + 1) * nb1].reshape(O_C, 64)),
            "qcode": np.ascontiguousarray(
                qcode[c * nb1:(c + 1) * nb1].reshape(O_C, 64)),
            "qoff": np.ascontiguousarray(
                qoff[c * nb1:(c + 1) * nb1].reshape(O_C, 64)),
            "am2": np.ascontiguousarray(
                am2[c * nb2:(c + 1) * nb2].reshape(O_C, 16)),
            "c2": np.ascontiguousarray(
                c2[c * nb2:(c + 1) * nb2].reshape(O_C, 16)),
        })
    return in_maps


def _run(inputs, trace=False, trace_cores=None):
    nc = _get_nc()
    in_maps = _shard(inputs)
    res = run_bass_kernel_spmd(
        nc, in_maps, list(range(N_CORES)), trace=trace,
        trace_cores=trace_cores)
    out = np.concatenate([r["out"] for r in res.results], axis=1)
    return out, res


def kernel(**inputs) -> np.ndarray:
    out, _ = _run(inputs, trace=False)
    return out


# revision 8
# speedup vs baseline: 1.0669x; 1.0575x over previous
"""NF4-style 4-bit quantized linear: out = x @ dequant(w).T on 8 TRN2 NeuronCores.

Column-parallel sharding: core c owns output features [c*512, (c+1)*512) and the
corresponding contiguous slices of the packed weight + quant state arrays; x is
replicated. All DMA traffic serializes against in-flight xbar transposes, so
the kernel is scheduled as one conveyor:
  1. first two packed-weight chunk loads (cast int32->int16 on the SWDGE
     queue) + scale state lead; the first x transpose is pinned after them so
     the dequant critical path never waits behind a 16us transpose,
  2. dequant runs on DVE in 8 chunks of 4 k-tiles (6 batched ops each,
     ~6.5us/chunk); w_nat/qt are triple-buffered so dequant free-runs ahead
     of the serialized store+transpose conveyor,
  3. each chunk round-trips through DRAM with an xbar transpose to reach
     wT [k-partition, outf] layout; the xbar chain is pinned
     [x0 w0 w1 w2 x1 w3 w4 w5 xt0 w6 w7 xt1 xt2 ...] over uniform 4MB
     (512-token) x blocks,
  4. the first two x blocks form the ramp (8 psums, chunk-major matmuls
     emitted in chain-readiness order); later blocks run k-serial.
A train of warmup matmuls on zeroed tiles keeps the PE busy from ~7us so the
HAM clock gate is warm when real matmuls start. Output stores are batched per
block; the last block stores per 128-token tile to shorten the drain.
"""
import numpy as np

import concourse.bass as bass
import concourse.mybir as mybir
import concourse.tile as tile
from concourse import bacc
from concourse.tile_rust import add_dep_helper as tile_rust_add_dep
from concourse.bass_utils import run_bass_kernel_spmd

F16 = mybir.dt.float16
F32 = mybir.dt.float32
I32 = mybir.dt.int32
I16 = mybir.dt.int16
Alu = mybir.AluOpType

P = 128
TOKENS = 8192
IN_F = 4096
OUT_F = 4096
N_CORES = 8
O_C = OUT_F // N_CORES          # 512 out features per core
KT = IN_F // P                  # 32 k-tiles
BPR = IN_F // 2                 # 2048 packed bytes per weight row
NB_O = O_C // P                 # 4 o-tiles of 128 rows
TB = 512                        # token block (4MB transpose)

NKC = 8                         # W chunks
KKC = KT // NKC                 # 4 k-tiles per chunk
KCW = IN_F // NKC               # 512 k values per chunk
BCC = BPR // NKC                # 256 packed bytes per chunk (per row)
NBC = BCC // 32                 # 8 quant blocks per chunk (per row)

NRB = 2                         # ramp blocks (2*512 tokens = 8 psums)

N_WARM = 96                     # warmup matmuls (~21us) to warm the HAM gate

# pinned xbar chain slot of each ramp x block / W chunk:
# [x0 w0 w1 w2 x1 w3 w4 w5 xt0 w6 w7 xt1 xt2 ...]
X_CH = [0, 4]
W_CH = [1, 2, 3, 5, 6, 7, 9, 10]
S_CH = [8, 11]                  # first two steady blocks' slots


def _build(tokens=TOKENS):
    nc = bacc.Bacc("TRN2", target_bir_lowering=False, debug=False,
                   enable_asserts=False)

    x = nc.dram_tensor("x", [tokens, IN_F], F16, kind="ExternalInput").ap()
    qw = nc.dram_tensor("qw", [O_C, BPR], I32, kind="ExternalInput").ap()
    qam = nc.dram_tensor("qam", [O_C, 64], I32, kind="ExternalInput").ap()
    qcode = nc.dram_tensor("qcode", [O_C, 64], F32, kind="ExternalInput").ap()
    qoff = nc.dram_tensor("qoff", [O_C, 64], F32, kind="ExternalInput").ap()
    am2 = nc.dram_tensor("am2", [O_C, 16], F32, kind="ExternalInput").ap()
    c2 = nc.dram_tensor("c2", [O_C, 16], F32, kind="ExternalInput").ap()
    out = nc.dram_tensor("out", [tokens, O_C], F16, kind="ExternalOutput").ap()

    n_steady = tokens // TB - NRB

    with tile.TileContext(nc) as tc:
        with tc.tile_pool(name="wt_pool", bufs=1) as wt_pool, \
             tc.tile_pool(name="wdram", bufs=1, space="DRAM") as wdram, \
             tc.tile_pool(name="sc_pool", bufs=1) as sc_pool, \
             tc.tile_pool(name="dq", bufs=2) as dq, \
             tc.tile_pool(name="xt_pool", bufs=2) as xt_pool, \
             tc.tile_pool(name="ps_pool", bufs=8, space="PSUM") as ps_pool, \
             tc.tile_pool(name="ob_pool", bufs=1) as ob_pool:

            # ---- SWDGE loads: chunk 0, scale state, chunk 1 lead; nothing
            # on the dequant critical path queues behind an xbar transpose.
            qw3 = qw.rearrange("(a p) c -> p a c", p=P)
            qts = {}

            def load_chunk(kc):
                qt = dq.tile([P, NB_O, BCC], I16, name="qt", bufs=3)
                li = nc.gpsimd.dma_start(
                    qt, qw3[:, :, kc * BCC:(kc + 1) * BCC])
                qts[kc] = qt
                return li

            load_chunk(0)
            am3 = sc_pool.tile([P, NB_O, 64], F32, name="am3")
            nc.gpsimd.dma_start(am3, qam.rearrange("(a p) c -> p a c", p=P))
            cd3 = sc_pool.tile([P, NB_O, 64], F32, name="cd3")
            nc.gpsimd.dma_start(cd3, qcode.rearrange("(a p) c -> p a c", p=P))
            c23 = sc_pool.tile([P, NB_O, 16], F32, name="c23")
            nc.gpsimd.dma_start(c23, c2.rearrange("(a p) c -> p a c", p=P))
            am23 = sc_pool.tile([P, NB_O, 16], F32, name="am23")
            nc.gpsimd.dma_start(am23, am2.rearrange("(a p) c -> p a c", p=P))
            of3 = sc_pool.tile([P, NB_O, 64], F32, name="of3")
            last_sc = nc.gpsimd.dma_start(
                of3, qoff.rearrange("(a p) c -> p a c", p=P))
            qw1_li = load_chunk(1)

            # ---- warmup matmuls on zeroed tiles: PE busy from ~7us so the
            # HAM clock gate is at 8/8 when real matmuls start.
            wz = sc_pool.tile([P, P], F16, name="wz")
            nc.vector.memset(wz, 0.0)
            ww = sc_pool.tile([P, O_C], F16, name="ww")
            nc.vector.memset(ww, 0.0)
            wps = ps_pool.tile([P, O_C], F32, name="ps")
            for _ in range(N_WARM):
                nc.tensor.matmul(wps, wz, ww, start=True, stop=True)

            # ---- scale prep (DVE):  S = (am/code) * (am2/c2) as fp16,
            # offS = off*S
            rc = sc_pool.tile([P, NB_O, 64], F32, name="rc")
            nc.vector.reciprocal_approx_fast(rc, cd3)
            s1 = sc_pool.tile([P, NB_O, 64], F32, name="s1")
            nc.vector.tensor_tensor(s1, am3, rc, Alu.mult)
            rc2 = sc_pool.tile([P, NB_O, 16], F32, name="rc2")
            nc.vector.reciprocal_approx_fast(rc2, c23)
            s2 = sc_pool.tile([P, NB_O, 16], F32, name="s2")
            nc.vector.tensor_tensor(s2, am23, rc2, Alu.mult)
            S3 = sc_pool.tile([P, NB_O, 64], F16, name="S3")
            nc.vector.tensor_tensor(
                S3, s1, s2.unsqueeze(3).broadcast_to([P, NB_O, 16, 4]), Alu.mult)
            offS3 = sc_pool.tile([P, NB_O, 64], F16, name="offS3")
            nc.vector.tensor_tensor(offS3, of3, S3, Alu.mult)

            # ---- dequant + W round-trip, chunk-major, batched ops ----
            wts = []
            wt_insts = []
            for kc in range(NKC):
                if kc + 2 < NKC:
                    load_chunk(kc + 2)
                wd = wdram.tile([O_C, KCW], F16, name=f"wd{kc}")
                w_nat = dq.tile([P, NB_O, KCW], F16, name="wn", bufs=3)
                qt = qts[kc]
                hi = dq.tile([P, NB_O, BCC], I16, name="hi", bufs=1)
                nc.vector.tensor_scalar(hi, qt, 4, None,
                                        Alu.logical_shift_right)
                lo = dq.tile([P, NB_O, BCC], F16, name="lo")
                nc.vector.scalar_tensor_tensor(
                    lo, hi, -16.0, qt, Alu.mult, Alu.add)
                sb = S3[:, :, kc * NBC:(kc + 1) * NBC] \
                    .unsqueeze(3).broadcast_to([P, NB_O, NBC, 32])
                mlo = dq.tile([P, NB_O, BCC], F16, name="mlo")
                nc.vector.tensor_tensor(mlo, lo, sb, Alu.mult)
                mhi = dq.tile([P, NB_O, BCC], F16, name="mhi")
                nc.vector.tensor_tensor(mhi, hi, sb, Alu.mult)
                offs = offS3[:, :, kc * NBC:(kc + 1) * NBC] \
                    .unsqueeze(3).broadcast_to([P, NB_O, NBC, 32])
                nc.vector.tensor_tensor(w_nat[:, :, 0::2], mlo, offs,
                                        Alu.subtract)
                nc.vector.tensor_tensor(w_nat[:, :, 1::2], mhi, offs,
                                        Alu.subtract)
                qts.pop(kc)
                nc.gpsimd.dma_start(
                    wd[:, :].rearrange("(a p) c -> p a c", p=P), w_nat)
                wt = wt_pool.tile([P, KKC, O_C], F16, name=f"wt{kc}")
                wi = nc.scalar.dma_start(out=wt, in_=wd[:, :], transpose=True)
                wts.append(wt)
                wt_insts.append(wi)

            # ---- ramp x transposes (4MB blocks of 512 tokens) ----
            xtr, xtr_insts = [], []
            for rb in range(NRB):
                t = xt_pool.tile([P, KT, TB], F16, name=f"xtr{rb}", bufs=1)
                ti = nc.scalar.dma_start(
                    out=t, in_=x[rb * TB:(rb + 1) * TB, :], transpose=True)
                xtr.append(t)
                xtr_insts.append(ti)

            # ---- ramp matmuls, emitted in chain-readiness order ----
            rps = [[ps_pool.tile([P, O_C], F32, name="ps")
                    for st in range(TB // P)] for rb in range(NRB)]
            groups = sorted(
                ((max(W_CH[kc], X_CH[rb]), kc, rb)
                 for kc in range(NKC) for rb in range(NRB)))
            for _, kc, rb in groups:
                for st in range(TB // P):
                    for j in range(KKC):
                        kk = kc * KKC + j
                        nc.tensor.matmul(
                            rps[rb][st],
                            xtr[rb][:, kk, st * P:(st + 1) * P],
                            wts[kc][:, j, :],
                            start=(kk == 0),
                            stop=(kk == KT - 1),
                        )
            for rb in range(NRB):
                ob = ob_pool.tile([P, TB // P, O_C], F16, name="ob")
                for st in range(TB // P):
                    nc.vector.tensor_copy(ob[:, st, :], rps[rb][st])
                r0 = rb * TB
                nc.gpsimd.dma_start(
                    out[r0:r0 + TB, :].rearrange("(st p) c -> p st c", p=P),
                    ob)

            # ---- steady blocks ----
            base = NRB * TB
            xt_insts = []
            for tb in range(n_steady):
                r0 = base + tb * TB
                xt = xt_pool.tile([P, KT, TB], F16, name="xt")
                xi = nc.scalar.dma_start(
                    out=xt, in_=x[r0:r0 + TB, :], transpose=True)
                xt_insts.append(xi)
                last = (tb == n_steady - 1)
                ob = ob_pool.tile([P, TB // P, O_C], F16, name="ob")
                for st in range(TB // P):
                    ps = ps_pool.tile([P, O_C], F32, name="ps")
                    for kk in range(KT):
                        nc.tensor.matmul(
                            ps,
                            xt[:, kk, st * P:(st + 1) * P],
                            wts[kk // KKC][:, kk % KKC, :],
                            start=(kk == 0),
                            stop=(kk == KT - 1),
                        )
                    nc.vector.tensor_copy(ob[:, st, :], ps)
                    if last:
                        # store per 128-token tile to shorten the drain
                        nc.gpsimd.dma_start(
                            out[r0 + st * P:r0 + (st + 1) * P, :],
                            ob[:, st, :])
                if not last:
                    nc.gpsimd.dma_start(
                        out[r0:r0 + TB, :].rearrange("(st p) c -> p st c", p=P),
                        ob)

            # ---- pin the xbar ring order; the first x block also waits for
            # the prologue loads so dequant inputs never queue behind it.
            chain = [None] * 12
            for rb in range(NRB):
                chain[X_CH[rb]] = xtr_insts[rb]
            for kc in range(NKC):
                chain[W_CH[kc]] = wt_insts[kc]
            chain[S_CH[0]] = xt_insts[0]
            chain[S_CH[1]] = xt_insts[1]
            chain += xt_insts[2:]
            for a, b in zip(chain[1:], chain):
                tile_rust_add_dep(a.ins, b.ins, True, "xbar order")
            tile_rust_add_dep(chain[0].ins, last_sc.ins, True, "scales first")
            tile_rust_add_dep(chain[0].ins, qw1_li.ins, True, "qw first")

    nc.compile()
    return nc


_NC_CACHE = {}


def _get_nc(tokens=TOKENS):
    if tokens not in _NC_CACHE:
        _NC_CACHE[tokens] = _build(tokens)
    return _NC_CACHE[tokens]


def _shard(inputs):
    x = np.ascontiguousarray(np.asarray(inputs["x"], dtype=np.float16))
    qw = np.asarray(inputs["quantized_weight"], dtype=np.int32)
    qam = np.asarray(inputs["quant_absmax"], dtype=np.int32)
    qcode = np.asarray(inputs["quant_code"], dtype=np.float32)
    qoff = np.asarray(inputs["quant_offset"], dtype=np.float32)
    am2 = np.asarray(inputs["state2_absmax"], dtype=np.float32)
    c2 = np.asarray(inputs["state2_code"], dtype=np.float32)

    pb = O_C * BPR        # packed bytes per core
    nb1 = O_C * 64        # primary blocks per core
    nb2 = O_C * 16        # secondary blocks per core
    in_maps = []
    for c in range(N_CORES):
        in_maps.append({
            "x": x,
            "qw": np.ascontiguousarray(
                qw[c * pb:(c + 1) * pb].reshape(O_C, BPR)),
            "qam": np.ascontiguousarray(
                qam[c * nb1:(c + 1) * nb1].reshape(O_C, 64)),
            "qcode": np.ascontiguousarray(
                qcode[c * nb1:(c + 1) * nb1].reshape(O_C, 64)),
            "qoff": np.ascontiguousarray(
                qoff[c * nb1:(c + 1) * nb1].reshape(O_C, 64)),
            "am2": np.ascontiguousarray(
                am2[c * nb2:(c + 1) * nb2].reshape(O_C, 16)),
            "c2": np.ascontiguousarray(
                c2[c * nb2:(c + 1) * nb2].reshape(O_C, 16)),
        })
    return in_maps


def _run(inputs, trace=False, trace_cores=None):
    nc = _get_nc()
    in_maps = _shard(inputs)
    res = run_bass_kernel_spmd(
        nc, in_maps, list(range(N_CORES)), trace=trace,
        trace_cores=trace_cores)
    out = np.concatenate([r["out"] for r in res.results], axis=1)
    return out, res


def kernel(**inputs) -> np.ndarray:
    out, _ = _run(inputs, trace=False)
    return out
